# revision 23
# baseline (speedup 1.0000x reference)
"""DirectionalRotationLoss Trainium2 kernel (8-core data-parallel).

Math (per quaternion pair p, t — both unnormalized, q = (w,x,y,z)):
  u(q) = n^2 * R(q_hat) e3 — scaled by 1/2 via pre-scaling the planes by
  1/sqrt(2) at deinterleave time:
     u1 = w'y' + x'z',  u2 = y'z' - w'x',
     s1 = w'^2 + z'^2,  s2 = x'^2 + y'^2,  u3 = s1 - s2,  n2 = s1 + s2
  (identically |u| == n2). d = <u_p, u_t>, g = n2_p * n2_t.
  theta = 2*atan(sqrt((g-d)/(g+d))) — tangent half-angle, no arccos.
  With m1 = min(g-d, g+d), m2 = max(g-d, g+d), phi = atan(sqrt(m1/m2)):
     theta^2 = 4*phi^2 + [d<0]*(pi^2 - 4*pi*phi)
  Mean over all pairs.

Engine split per tile (bf16 compute):
  ScalarE: 8 strided deinterleave+convert+scale copies, Sqrt, Square+accum
           (copy/square/sqrt all live in the sqrt_and_others table set —
           exactly one ACT table load for the whole kernel).
  VectorE: quadratics as stock bf16 tensor_tensor (2x mode) plus three
           runtime-registered custom DVE ops: SQSUM (sq+sq), ATAN_ODD7
           (odd polynomial in t), CONDACC ([d<0]-correction with free-dim
           accumulation), and the stock RECIPROCAL_APPROX_FAST.
  Per-partition partial sums land in two [128, NIT] f32 accumulators,
  reduced on the host (x4 and the pi^2-correction applied there).
"""

import numpy as np
from operator import add as _op_add

import concourse.bass as bass
import concourse.bacc as bacc
import concourse.mybir as mybir
from concourse.tile import TileContext
from concourse.bass_utils import run_bass_kernel_spmd
from concourse.dve_spec import Spec, Src0, Src1, C0, C1, C2, Zero, One, lower, sq, _has_src1
from concourse.dve_uop import DveOpSpec
import concourse.dve_ops as dve_ops
from concourse.dve_ops import (
    DveOp,
    OPS,
    get_dve_sub_opcode,
    RECIPROCAL_APPROX_FAST,
    RECIP_APPROX_FAST_CONSTS,
)

NCORES = 8
P = 128
B = 8388608
QPC = B // NCORES          # quats per core
QPP = QPC // P             # quats per partition (8192)
M = 2048                   # quats per partition per iteration
NIT = QPP // M             # iterations (4)

F32 = mybir.dt.float32
BF16 = mybir.dt.bfloat16
AL = mybir.AluOpType
AF = mybir.ActivationFunctionType

HALF_SCALE = float(1.0 / np.sqrt(2.0))
TINY = 1e-6
PI = float(np.pi)

# atan(t) ~= t*(1 + t^2*(A1 + A2 t^2 + A3 t^4)) on [0,1] (a0 pinned to 1 so
# the poly fits the 3 scalar slots of one custom DVE op); max err ~8.8e-4 rad,
# below the bf16 rounding already present.
A1, A2, A3 = -0.33215468080413524, 0.17587160443286803, -0.059202957570287394


def _make_op(name, spec, subdim=False):
    for op in OPS:
        if op.name == name:
            return op
    shas = {}
    op = DveOp(name, spec, subdim=subdim, uops_sha=shas)
    OPS.append(op)
    dve_ops.CUSTOM_DVE_SPECS[name] = spec
    dve_ops._SUB_OPCODE_FOR_NAME[name] = dve_ops._CUSTOM_DVE_ROW_BASE + len(OPS) - 1
    for ver in ("v3", "v4"):
        r = DveOpSpec(
            name=name,
            opcode=get_dve_sub_opcode(name),
            uops=lower(spec, ver=ver),
            rd1_en=_has_src1(spec),
        )
        shas[ver] = r.sha(ver)
    return op


_t2 = sq(Src0)
ATAN7 = _make_op(
    "ATAN_ODD7_ANT",
    Spec(
        body=(((C2 * _t2 + C1) * _t2 + C0) * _t2 + One) * Src0,
        reference=lambda in0, in1, s0, s1, imm2: (
            ((imm2 * in0 * in0 + s1) * in0 * in0 + s0) * in0 * in0 + 1.0
        )
        * in0,
    ),
)
CONDACC = _make_op(
    "CONDACC_ANT",
    Spec(
        body=(Src0 < Zero) * (C0 - C1 * Src1),
        accum=_op_add,
        accum_init=Zero,
        reference=lambda in0, in1, s0, s1, imm2: (in0 < 0) * (s0 - s1 * in1),
    ),
)
from concourse.dve_spec import Bin, AluOp as _AluOp, maxx as _maxx, minn as _minn
_sub = Src0 - Src1
_add2 = Src0 + Src1
M1C = _make_op(
    "M1CLAMP_ANT",
    Spec(
        body=_maxx(_minn(_sub, _add2), C0),
        reference=lambda in0, in1, s0, s1, imm2: np.maximum(
            np.minimum(in0 - in1, in0 + in1), s0
        ),
    ),
)
M2C = _make_op(
    "M2MAX_ANT",
    Spec(
        body=_maxx(_sub, _add2),
        reference=lambda in0, in1, s0, s1, imm2: np.maximum(in0 - in1, in0 + in1),
    ),
)
_not_m2 = Bin(_AluOp.BITWISE_NOT, Src0, Src0)
_y0 = _not_m2 * C0
_y1 = _y0 * (C1 - Src0 * _y0)
RHORECIP = _make_op(
    "RHORECIP_ANT",
    Spec(
        body=_y1 * Src1,
        reference=lambda in0, in1, s0, s1, imm2: (
            lambda y0: (y0 * (s1 - in0 * y0)) * in1
        )((~in0.view(np.int32)).view(np.float32) * s0),
    ),
)
SQSUM = _make_op(
    "SQSUMS_ANT",
    Spec(
        body=(sq(Src0) + sq(Src1)) * C0,
        reference=lambda in0, in1, s0, s1, imm2: (in0 * in0 + in1 * in1) * s0,
    ),
)


def _emit(nc, reps=1):
    pred = nc.declare_dram_parameter("pred", [P, QPP * 4], F32, isOutput=False)
    targ = nc.declare_dram_parameter("target", [P, QPP * 4], F32, isOutput=False)
    out = nc.declare_dram_parameter("out", [P, 2 * NIT], F32, isOutput=True)
    rc = RECIP_APPROX_FAST_CONSTS

    with TileContext(nc) as tc:
        with (
            tc.tile_pool(name="stg", bufs=2) as stg,
            tc.tile_pool(name="pl", bufs=10) as plp,
            tc.tile_pool(name="u", bufs=11) as up,
            tc.tile_pool(name="tmp", bufs=13) as tp,
            tc.tile_pool(name="jk", bufs=2) as jkp,
            tc.tile_pool(name="st", bufs=1) as stp,
        ):
            stats1 = stp.tile([P, NIT], F32, tag="s1", name="stats1")
            stats2 = stp.tile([P, NIT], F32, tag="s2", name="stats2")

            import contextlib

            loop_cm = tc.For_i(0, reps, 1) if reps > 1 else contextlib.nullcontext()
            with loop_cm:
              for it in range(NIT):
                us = {}
                dsl = slice(it * 4 * M, (it + 1) * 4 * M)
                for side, par in (("p", pred), ("t", targ)):
                    T = stg.tile([P, 4 * M], F32, tag="stage", name=f"T{side}{it}")
                    nc.sync.dma_start(out=T[:, :], in_=par[:, dsl])

                    # deinterleave + convert + pre-scale by 1/sqrt(2)
                    pls = []
                    for c in range(4):
                        pt = plp.tile([P, M], BF16, tag="pl", name=f"pl{c}{side}{it}")
                        nc.vector.tensor_copy(pt[:, :], T[:, c::4])
                        pls.append(pt)
                    w, x, y, z = pls

                    s1 = tp.tile([P, M], BF16, tag="tmp", name=f"s1{side}{it}")
                    s2 = tp.tile([P, M], BF16, tag="tmp", name=f"s2{side}{it}")
                    nc.vector._custom_dve(SQSUM, out=s1[:, :], in0=w[:, :], in1=z[:, :], s0=0.5)
                    nc.vector._custom_dve(SQSUM, out=s2[:, :], in0=x[:, :], in1=y[:, :], s0=0.5)
                    u3 = up.tile([P, M], BF16, tag="u", name=f"u3{side}{it}")
                    n2 = up.tile([P, M], BF16, tag="u", name=f"n2{side}{it}")
                    nc.vector.tensor_sub(u3[:, :], s1[:, :], s2[:, :])
                    nc.vector.tensor_add(n2[:, :], s1[:, :], s2[:, :])

                    wy = tp.tile([P, M], BF16, tag="tmp", name=f"wy{side}{it}")
                    xz = tp.tile([P, M], BF16, tag="tmp", name=f"xz{side}{it}")
                    yz = tp.tile([P, M], BF16, tag="tmp", name=f"yz{side}{it}")
                    wx = tp.tile([P, M], BF16, tag="tmp", name=f"wx{side}{it}")
                    nc.vector.tensor_mul(wy[:, :], w[:, :], y[:, :])
                    nc.vector.tensor_mul(xz[:, :], x[:, :], z[:, :])
                    nc.vector.tensor_mul(yz[:, :], y[:, :], z[:, :])
                    nc.vector.tensor_mul(wx[:, :], w[:, :], x[:, :])
                    u1 = up.tile([P, M], BF16, tag="u", name=f"u1{side}{it}")
                    u2 = up.tile([P, M], BF16, tag="u", name=f"u2{side}{it}")
                    nc.vector.tensor_add(u1[:, :], wy[:, :], xz[:, :])
                    nc.vector.tensor_sub(u2[:, :], yz[:, :], wx[:, :])
                    us[side] = (u1, u2, u3, n2)

                u1p, u2p, u3p, n2p = us["p"]
                u1t, u2t, u3t, n2t = us["t"]

                g = tp.tile([P, M], BF16, tag="tmp", name=f"g{it}")
                d1 = tp.tile([P, M], BF16, tag="tmp", name=f"d1{it}")
                d2 = tp.tile([P, M], BF16, tag="tmp", name=f"d2{it}")
                d3 = tp.tile([P, M], BF16, tag="tmp", name=f"d3{it}")
                nc.vector.tensor_mul(g[:, :], n2p[:, :], n2t[:, :])
                nc.vector.tensor_mul(d1[:, :], u1p[:, :], u1t[:, :])
                nc.vector.tensor_mul(d2[:, :], u2p[:, :], u2t[:, :])
                nc.vector.tensor_mul(d3[:, :], u3p[:, :], u3t[:, :])
                dd = tp.tile([P, M], BF16, tag="tmp", name=f"dd{it}")
                d = tp.tile([P, M], BF16, tag="tmp", name=f"d{it}")
                nc.vector.tensor_add(dd[:, :], d1[:, :], d2[:, :])
                nc.vector.tensor_add(d[:, :], dd[:, :], d3[:, :])

                m1 = tp.tile([P, M], BF16, tag="tmp", name=f"m1{it}")
                m2 = tp.tile([P, M], BF16, tag="tmp", name=f"m2{it}")
                nc.vector._custom_dve(M1C, out=m1[:, :], in0=g[:, :], in1=d[:, :], s0=TINY)
                nc.vector._custom_dve(M2C, out=m2[:, :], in0=g[:, :], in1=d[:, :])
                rho = tp.tile([P, M], BF16, tag="tmp", name=f"rho{it}")
                nc.vector._custom_dve(
                    RHORECIP, out=rho[:, :], in0=m2[:, :], in1=m1[:, :],
                    s0=rc["s0"], s1=rc["s1"],
                )
                t = tp.tile([P, M], BF16, tag="tmp", name=f"t{it}")
                nc.scalar.activation(t[:, :], rho[:, :], AF.Sqrt)

                phi = tp.tile([P, M], BF16, tag="tmp", name=f"phi{it}")
                nc.vector._custom_dve(
                    ATAN7, out=phi[:, :], in0=t[:, :], s0=A1, s1=A2, imm2=A3
                )

                # stats1[:, it] = sum(phi^2)
                ph2 = jkp.tile([P, M], BF16, tag="jk", name=f"ph2{it}")
                nc.scalar.activation(
                    ph2[:, :], phi[:, :], AF.Square,
                    accum_out=stats1[:, it : it + 1],
                )
                # stats2[:, it] = sum([d<0] * (pi^2 - 4*pi*phi))
                cjk = jkp.tile([P, M], BF16, tag="jk", name=f"cjk{it}")
                nc.vector._custom_dve(
                    CONDACC,
                    out=cjk[:, :],
                    accum_out=stats2[:, it : it + 1],
                    in0=d[:, :],
                    in1=phi[:, :],
                    s0=PI * PI,
                    s1=4.0 * PI,
                )

            nc.sync.dma_start(out=out[:, 0:NIT], in_=stats1[:, :])
            nc.sync.dma_start(out=out[:, NIT : 2 * NIT], in_=stats2[:, :])
    return nc


_CACHE = {}


def _get_nc(reps=1):
    key = ("nc", reps)
    if key not in _CACHE:
        nc = _emit(bacc.Bacc(), reps=reps)
        nc.compile()
        _CACHE[key] = nc
    return _CACHE[key]


def kernel(pred: np.ndarray, target: np.ndarray) -> np.ndarray:
    pred = np.ascontiguousarray(pred, dtype=np.float32)
    target = np.ascontiguousarray(target, dtype=np.float32)
    assert pred.shape == (B, 4) and target.shape == (B, 4)

    nc = _get_nc()
    in_maps = []
    for c in range(NCORES):
        sl = slice(c * QPC, (c + 1) * QPC)
        in_maps.append(
            {
                "pred": pred[sl].reshape(P, QPP * 4),
                "target": target[sl].reshape(P, QPP * 4),
            }
        )
    res = run_bass_kernel_spmd(nc, in_maps, list(range(NCORES)))
    total = 0.0
    for r in res.results:
        o = np.asarray(r["out"], np.float64)
        total += 4.0 * o[:, 0:NIT].sum() + o[:, NIT:].sum()
    return np.float32(total / B)


# revision 24
# speedup vs baseline: 1.0780x; 1.0780x over previous
"""DirectionalRotationLoss Trainium2 kernel (8-core data-parallel).

Math (per quaternion pair p, t — both unnormalized, q = (w,x,y,z)):
  u(q) = n^2 * R(q_hat) e3 — scaled by 1/2 via pre-scaling the planes by
  1/sqrt(2) at deinterleave time:
     u1 = w'y' + x'z',  u2 = y'z' - w'x',
     s1 = w'^2 + z'^2,  s2 = x'^2 + y'^2,  u3 = s1 - s2,  n2 = s1 + s2
  (identically |u| == n2). d = <u_p, u_t>, g = n2_p * n2_t.
  theta = 2*atan(sqrt((g-d)/(g+d))) — tangent half-angle, no arccos.
  With m1 = min(g-d, g+d), m2 = max(g-d, g+d), phi = atan(sqrt(m1/m2)):
     theta^2 = 4*phi^2 + [d<0]*(pi^2 - 4*pi*phi)
  Mean over all pairs.

Engine split per tile (bf16 compute):
  ScalarE: 8 strided deinterleave+convert+scale copies, Sqrt, Square+accum
           (copy/square/sqrt all live in the sqrt_and_others table set —
           exactly one ACT table load for the whole kernel).
  VectorE: quadratics as stock bf16 tensor_tensor (2x mode) plus three
           runtime-registered custom DVE ops: SQSUM (sq+sq), ATAN_ODD7
           (odd polynomial in t), CONDACC ([d<0]-correction with free-dim
           accumulation), and the stock RECIPROCAL_APPROX_FAST.
  Per-partition partial sums land in two [128, NIT] f32 accumulators,
  reduced on the host (x4 and the pi^2-correction applied there).
"""

import numpy as np
from operator import add as _op_add

import concourse.bass as bass
import concourse.bacc as bacc
import concourse.mybir as mybir
from concourse.tile import TileContext
from concourse.bass_utils import run_bass_kernel_spmd
from concourse.dve_spec import Spec, Src0, Src1, C0, C1, C2, Zero, One, lower, sq, _has_src1
from concourse.dve_uop import DveOpSpec
import concourse.dve_ops as dve_ops
from concourse.dve_ops import (
    DveOp,
    OPS,
    get_dve_sub_opcode,
    RECIPROCAL_APPROX_FAST,
    RECIP_APPROX_FAST_CONSTS,
)

NCORES = 8
P = 128
B = 8388608
QPC = B // NCORES          # quats per core
QPP = QPC // P             # quats per partition (8192)
M = 2048                   # quats per partition per iteration
NIT = QPP // M             # iterations (4)

F32 = mybir.dt.float32
BF16 = mybir.dt.bfloat16
AL = mybir.AluOpType
AF = mybir.ActivationFunctionType

HALF_SCALE = float(1.0 / np.sqrt(2.0))
TINY = 1e-6
PI = float(np.pi)

# atan(t) ~= t*(1 + t^2*(A1 + A2 t^2 + A3 t^4)) on [0,1] (a0 pinned to 1 so
# the poly fits the 3 scalar slots of one custom DVE op); max err ~8.8e-4 rad,
# below the bf16 rounding already present.
A1, A2, A3 = -0.33215468080413524, 0.17587160443286803, -0.059202957570287394


def _make_op(name, spec, subdim=False):
    for op in OPS:
        if op.name == name:
            return op
    shas = {}
    op = DveOp(name, spec, subdim=subdim, uops_sha=shas)
    OPS.append(op)
    dve_ops.CUSTOM_DVE_SPECS[name] = spec
    dve_ops._SUB_OPCODE_FOR_NAME[name] = dve_ops._CUSTOM_DVE_ROW_BASE + len(OPS) - 1
    for ver in ("v3", "v4"):
        r = DveOpSpec(
            name=name,
            opcode=get_dve_sub_opcode(name),
            uops=lower(spec, ver=ver),
            rd1_en=_has_src1(spec),
        )
        shas[ver] = r.sha(ver)
    return op


_t2 = sq(Src0)
ATAN7 = _make_op(
    "ATAN_ODD7_ANT",
    Spec(
        body=(((C2 * _t2 + C1) * _t2 + C0) * _t2 + One) * Src0,
        reference=lambda in0, in1, s0, s1, imm2: (
            ((imm2 * in0 * in0 + s1) * in0 * in0 + s0) * in0 * in0 + 1.0
        )
        * in0,
    ),
)
CONDACC = _make_op(
    "CONDACC_ANT",
    Spec(
        body=(Src0 < Zero) * (C0 - C1 * Src1),
        accum=_op_add,
        accum_init=Zero,
        reference=lambda in0, in1, s0, s1, imm2: (in0 < 0) * (s0 - s1 * in1),
    ),
)
from concourse.dve_spec import Bin, AluOp as _AluOp, maxx as _maxx, minn as _minn
_sub = Src0 - Src1
_add2 = Src0 + Src1
M1C = _make_op(
    "M1CLAMP_ANT",
    Spec(
        body=_maxx(_minn(_sub, _add2), C0),
        reference=lambda in0, in1, s0, s1, imm2: np.maximum(
            np.minimum(in0 - in1, in0 + in1), s0
        ),
    ),
)
M2C = _make_op(
    "M2MAX_ANT",
    Spec(
        body=_maxx(_sub, _add2),
        reference=lambda in0, in1, s0, s1, imm2: np.maximum(in0 - in1, in0 + in1),
    ),
)
_not_m2 = Bin(_AluOp.BITWISE_NOT, Src0, Src0)
_y0 = _not_m2 * C0
_y1 = _y0 * (C1 - Src0 * _y0)
RHORECIP = _make_op(
    "RHORECIP_ANT",
    Spec(
        body=_y1 * Src1,
        reference=lambda in0, in1, s0, s1, imm2: (
            lambda y0: (y0 * (s1 - in0 * y0)) * in1
        )((~in0.view(np.int32)).view(np.float32) * s0),
    ),
)
SQSUM = _make_op(
    "SQSUMS_ANT",
    Spec(
        body=(sq(Src0) + sq(Src1)) * C0,
        reference=lambda in0, in1, s0, s1, imm2: (in0 * in0 + in1 * in1) * s0,
    ),
)


def _emit(nc, reps=1):
    pred = nc.declare_dram_parameter("pred", [P, QPP * 4], F32, isOutput=False)
    targ = nc.declare_dram_parameter("target", [P, QPP * 4], F32, isOutput=False)
    out = nc.declare_dram_parameter("out", [P, 2 * NIT], F32, isOutput=True)
    rc = RECIP_APPROX_FAST_CONSTS

    with TileContext(nc) as tc:
        with (
            tc.tile_pool(name="stg", bufs=2) as stg,
            tc.tile_pool(name="pl", bufs=9) as plp,
            tc.tile_pool(name="u", bufs=12) as up,
            tc.tile_pool(name="tmp", bufs=12) as tp,
            tc.tile_pool(name="jk", bufs=2) as jkp,
            tc.tile_pool(name="st", bufs=1) as stp,
        ):
            stats1 = stp.tile([P, NIT], F32, tag="s1", name="stats1")
            stats2 = stp.tile([P, NIT], F32, tag="s2", name="stats2")

            import contextlib

            loop_cm = tc.For_i(0, reps, 1) if reps > 1 else contextlib.nullcontext()
            with loop_cm:
              for it in range(NIT):
                us = {}
                dsl = slice(it * 4 * M, (it + 1) * 4 * M)
                for side, par in (("p", pred), ("t", targ)):
                    T = stg.tile([P, 4 * M], F32, tag="stage", name=f"T{side}{it}")
                    nc.sync.dma_start(out=T[:, :], in_=par[:, dsl])

                    # deinterleave + convert + pre-scale by 1/sqrt(2)
                    pls = []
                    for c in range(4):
                        pt = plp.tile([P, M], BF16, tag="pl", name=f"pl{c}{side}{it}")
                        nc.vector.tensor_copy(pt[:, :], T[:, c::4])
                        pls.append(pt)
                    w, x, y, z = pls

                    s1 = tp.tile([P, M], BF16, tag="tmp", name=f"s1{side}{it}")
                    s2 = tp.tile([P, M], BF16, tag="tmp", name=f"s2{side}{it}")
                    nc.vector._custom_dve(SQSUM, out=s1[:, :], in0=w[:, :], in1=z[:, :], s0=0.5)
                    nc.vector._custom_dve(SQSUM, out=s2[:, :], in0=x[:, :], in1=y[:, :], s0=0.5)
                    u3 = up.tile([P, M], BF16, tag="u", name=f"u3{side}{it}")
                    n2 = up.tile([P, M], BF16, tag="u", name=f"n2{side}{it}")
                    nc.vector.tensor_sub(u3[:, :], s1[:, :], s2[:, :])
                    nc.vector.tensor_add(n2[:, :], s1[:, :], s2[:, :])

                    wy = tp.tile([P, M], BF16, tag="tmp", name=f"wy{side}{it}")
                    xz = tp.tile([P, M], BF16, tag="tmp", name=f"xz{side}{it}")
                    yz = tp.tile([P, M], BF16, tag="tmp", name=f"yz{side}{it}")
                    wx = tp.tile([P, M], BF16, tag="tmp", name=f"wx{side}{it}")
                    nc.vector.tensor_mul(wy[:, :], w[:, :], y[:, :])
                    nc.vector.tensor_mul(xz[:, :], x[:, :], z[:, :])
                    nc.vector.tensor_mul(yz[:, :], y[:, :], z[:, :])
                    nc.vector.tensor_mul(wx[:, :], w[:, :], x[:, :])
                    u1 = up.tile([P, M], BF16, tag="u", name=f"u1{side}{it}")
                    u2 = up.tile([P, M], BF16, tag="u", name=f"u2{side}{it}")
                    nc.vector.tensor_add(u1[:, :], wy[:, :], xz[:, :])
                    nc.vector.tensor_sub(u2[:, :], yz[:, :], wx[:, :])
                    us[side] = (u1, u2, u3, n2)

                u1p, u2p, u3p, n2p = us["p"]
                u1t, u2t, u3t, n2t = us["t"]

                g = tp.tile([P, M], BF16, tag="tmp", name=f"g{it}")
                d1 = tp.tile([P, M], BF16, tag="tmp", name=f"d1{it}")
                d2 = tp.tile([P, M], BF16, tag="tmp", name=f"d2{it}")
                d3 = tp.tile([P, M], BF16, tag="tmp", name=f"d3{it}")
                nc.vector.tensor_mul(g[:, :], n2p[:, :], n2t[:, :])
                nc.vector.tensor_mul(d1[:, :], u1p[:, :], u1t[:, :])
                nc.vector.tensor_mul(d2[:, :], u2p[:, :], u2t[:, :])
                nc.vector.tensor_mul(d3[:, :], u3p[:, :], u3t[:, :])
                dd = tp.tile([P, M], BF16, tag="tmp", name=f"dd{it}")
                d = tp.tile([P, M], BF16, tag="tmp", name=f"d{it}")
                nc.vector.tensor_add(dd[:, :], d1[:, :], d2[:, :])
                nc.vector.tensor_add(d[:, :], dd[:, :], d3[:, :])

                m1 = tp.tile([P, M], BF16, tag="tmp", name=f"m1{it}")
                m2 = tp.tile([P, M], BF16, tag="tmp", name=f"m2{it}")
                nc.vector._custom_dve(M1C, out=m1[:, :], in0=g[:, :], in1=d[:, :], s0=TINY)
                nc.vector._custom_dve(M2C, out=m2[:, :], in0=g[:, :], in1=d[:, :])
                rho = tp.tile([P, M], BF16, tag="tmp", name=f"rho{it}")
                nc.vector._custom_dve(
                    RHORECIP, out=rho[:, :], in0=m2[:, :], in1=m1[:, :],
                    s0=rc["s0"], s1=rc["s1"],
                )
                t = tp.tile([P, M], BF16, tag="tmp", name=f"t{it}")
                nc.scalar.activation(t[:, :], rho[:, :], AF.Sqrt)

                phi = tp.tile([P, M], BF16, tag="tmp", name=f"phi{it}")
                nc.vector._custom_dve(
                    ATAN7, out=phi[:, :], in0=t[:, :], s0=A1, s1=A2, imm2=A3
                )

                # stats1[:, it] = sum(phi^2)
                ph2 = jkp.tile([P, M], BF16, tag="jk", name=f"ph2{it}")
                nc.scalar.activation(
                    ph2[:, :], phi[:, :], AF.Square,
                    accum_out=stats1[:, it : it + 1],
                )
                # stats2[:, it] = sum([d<0] * (pi^2 - 4*pi*phi))
                cjk = jkp.tile([P, M], BF16, tag="jk", name=f"cjk{it}")
                nc.vector._custom_dve(
                    CONDACC,
                    out=cjk[:, :],
                    accum_out=stats2[:, it : it + 1],
                    in0=d[:, :],
                    in1=phi[:, :],
                    s0=PI * PI,
                    s1=4.0 * PI,
                )

            nc.sync.dma_start(out=out[:, 0:NIT], in_=stats1[:, :])
            nc.sync.dma_start(out=out[:, NIT : 2 * NIT], in_=stats2[:, :])
    return nc


_CACHE = {}


def _get_nc(reps=1):
    key = ("nc", reps)
    if key not in _CACHE:
        nc = _emit(bacc.Bacc(), reps=reps)
        nc.compile()
        _CACHE[key] = nc
    return _CACHE[key]


def kernel(pred: np.ndarray, target: np.ndarray) -> np.ndarray:
    pred = np.ascontiguousarray(pred, dtype=np.float32)
    target = np.ascontiguousarray(target, dtype=np.float32)
    assert pred.shape == (B, 4) and target.shape == (B, 4)

    nc = _get_nc()
    in_maps = []
    for c in range(NCORES):
        sl = slice(c * QPC, (c + 1) * QPC)
        in_maps.append(
            {
                "pred": pred[sl].reshape(P, QPP * 4),
                "target": target[sl].reshape(P, QPP * 4),
            }
        )
    res = run_bass_kernel_spmd(nc, in_maps, list(range(NCORES)))
    total = 0.0
    for r in res.results:
        o = np.asarray(r["out"], np.float64)
        total += 4.0 * o[:, 0:NIT].sum() + o[:, NIT:].sum()
    return np.float32(total / B)


# revision 25
# speedup vs baseline: 1.0869x; 1.0082x over previous
"""DirectionalRotationLoss Trainium2 kernel (8-core data-parallel).

Math (per quaternion pair p, t — both unnormalized, q = (w,x,y,z)):
  u(q) = n^2 * R(q_hat) e3 — scaled by 1/2 via pre-scaling the planes by
  1/sqrt(2) at deinterleave time:
     u1 = w'y' + x'z',  u2 = y'z' - w'x',
     s1 = w'^2 + z'^2,  s2 = x'^2 + y'^2,  u3 = s1 - s2,  n2 = s1 + s2
  (identically |u| == n2). d = <u_p, u_t>, g = n2_p * n2_t.
  theta = 2*atan(sqrt((g-d)/(g+d))) — tangent half-angle, no arccos.
  With m1 = min(g-d, g+d), m2 = max(g-d, g+d), phi = atan(sqrt(m1/m2)):
     theta^2 = 4*phi^2 + [d<0]*(pi^2 - 4*pi*phi)
  Mean over all pairs.

Engine split per tile (bf16 compute):
  ScalarE: 8 strided deinterleave+convert+scale copies, Sqrt, Square+accum
           (copy/square/sqrt all live in the sqrt_and_others table set —
           exactly one ACT table load for the whole kernel).
  VectorE: quadratics as stock bf16 tensor_tensor (2x mode) plus three
           runtime-registered custom DVE ops: SQSUM (sq+sq), ATAN_ODD7
           (odd polynomial in t), CONDACC ([d<0]-correction with free-dim
           accumulation), and the stock RECIPROCAL_APPROX_FAST.
  Per-partition partial sums land in two [128, NIT] f32 accumulators,
  reduced on the host (x4 and the pi^2-correction applied there).
"""

import numpy as np
from operator import add as _op_add

import concourse.bass as bass
import concourse.bacc as bacc
import concourse.mybir as mybir
from concourse.tile import TileContext
from concourse.bass_utils import run_bass_kernel_spmd
from concourse.dve_spec import Spec, Src0, Src1, C0, C1, C2, Zero, One, lower, sq, _has_src1
from concourse.dve_uop import DveOpSpec
import concourse.dve_ops as dve_ops
from concourse.dve_ops import (
    DveOp,
    OPS,
    get_dve_sub_opcode,
    RECIPROCAL_APPROX_FAST,
    RECIP_APPROX_FAST_CONSTS,
)

NCORES = 8
P = 128
B = 8388608
QPC = B // NCORES          # quats per core
QPP = QPC // P             # quats per partition (8192)
M = 2048                   # quats per partition per iteration
NIT = QPP // M             # iterations (4)

F32 = mybir.dt.float32
BF16 = mybir.dt.bfloat16
AL = mybir.AluOpType
AF = mybir.ActivationFunctionType

HALF_SCALE = float(1.0 / np.sqrt(2.0))
TINY = 1e-6
PI = float(np.pi)

# atan(t) ~= t*(1 + t^2*(A1 + A2 t^2 + A3 t^4)) on [0,1] (a0 pinned to 1 so
# the poly fits the 3 scalar slots of one custom DVE op); max err ~8.8e-4 rad,
# below the bf16 rounding already present.
A1, A2, A3 = -0.33215468080413524, 0.17587160443286803, -0.059202957570287394


def _make_op(name, spec, subdim=False):
    for op in OPS:
        if op.name == name:
            return op
    shas = {}
    op = DveOp(name, spec, subdim=subdim, uops_sha=shas)
    OPS.append(op)
    dve_ops.CUSTOM_DVE_SPECS[name] = spec
    dve_ops._SUB_OPCODE_FOR_NAME[name] = dve_ops._CUSTOM_DVE_ROW_BASE + len(OPS) - 1
    for ver in ("v3", "v4"):
        r = DveOpSpec(
            name=name,
            opcode=get_dve_sub_opcode(name),
            uops=lower(spec, ver=ver),
            rd1_en=_has_src1(spec),
        )
        shas[ver] = r.sha(ver)
    return op


_t2 = sq(Src0)
ATAN7 = _make_op(
    "ATAN_ODD7_ANT",
    Spec(
        body=(((C2 * _t2 + C1) * _t2 + C0) * _t2 + One) * Src0,
        reference=lambda in0, in1, s0, s1, imm2: (
            ((imm2 * in0 * in0 + s1) * in0 * in0 + s0) * in0 * in0 + 1.0
        )
        * in0,
    ),
)
CONDACC = _make_op(
    "CONDACC_ANT",
    Spec(
        body=(Src0 < Zero) * (C0 - C1 * Src1),
        accum=_op_add,
        accum_init=Zero,
        reference=lambda in0, in1, s0, s1, imm2: (in0 < 0) * (s0 - s1 * in1),
    ),
)
from concourse.dve_spec import Bin, AluOp as _AluOp, maxx as _maxx, minn as _minn
_sub = Src0 - Src1
_add2 = Src0 + Src1
M1C = _make_op(
    "M1CLAMP_ANT",
    Spec(
        body=_maxx(_minn(_sub, _add2), C0),
        reference=lambda in0, in1, s0, s1, imm2: np.maximum(
            np.minimum(in0 - in1, in0 + in1), s0
        ),
    ),
)
M2C = _make_op(
    "M2MAX_ANT",
    Spec(
        body=_maxx(_sub, _add2),
        reference=lambda in0, in1, s0, s1, imm2: np.maximum(in0 - in1, in0 + in1),
    ),
)
_not_m2 = Bin(_AluOp.BITWISE_NOT, Src0, Src0)
_y0 = _not_m2 * C0
_y1 = _y0 * (C1 - Src0 * _y0)
RHORECIP = _make_op(
    "RHORECIP_ANT",
    Spec(
        body=_y1 * Src1,
        reference=lambda in0, in1, s0, s1, imm2: (
            lambda y0: (y0 * (s1 - in0 * y0)) * in1
        )((~in0.view(np.int32)).view(np.float32) * s0),
    ),
)
SQSUM = _make_op(
    "SQSUMS_ANT",
    Spec(
        body=(sq(Src0) + sq(Src1)) * C0,
        reference=lambda in0, in1, s0, s1, imm2: (in0 * in0 + in1 * in1) * s0,
    ),
)


def _emit(nc, reps=1):
    pred = nc.declare_dram_parameter("pred", [P, QPP * 4], F32, isOutput=False)
    targ = nc.declare_dram_parameter("target", [P, QPP * 4], F32, isOutput=False)
    out = nc.declare_dram_parameter("out", [P, 2 * NIT], F32, isOutput=True)
    rc = RECIP_APPROX_FAST_CONSTS

    with TileContext(nc) as tc:
        with (
            tc.tile_pool(name="stg", bufs=3) as stg,
            tc.tile_pool(name="pl", bufs=8) as plp,
            tc.tile_pool(name="u", bufs=9) as up,
            tc.tile_pool(name="tmp", bufs=8) as tp,
            tc.tile_pool(name="jk", bufs=2) as jkp,
            tc.tile_pool(name="st", bufs=1) as stp,
        ):
            stats1 = stp.tile([P, NIT], F32, tag="s1", name="stats1")
            stats2 = stp.tile([P, NIT], F32, tag="s2", name="stats2")

            import contextlib

            loop_cm = tc.For_i(0, reps, 1) if reps > 1 else contextlib.nullcontext()
            with loop_cm:
              for it in range(NIT):
                us = {}
                dsl = slice(it * 4 * M, (it + 1) * 4 * M)
                for side, par in (("p", pred), ("t", targ)):
                    T = stg.tile([P, 4 * M], F32, tag="stage", name=f"T{side}{it}")
                    nc.sync.dma_start(out=T[:, :], in_=par[:, dsl])

                    # deinterleave + convert + pre-scale by 1/sqrt(2)
                    pls = []
                    for c in range(4):
                        pt = plp.tile([P, M], BF16, tag="pl", name=f"pl{c}{side}{it}")
                        nc.vector.tensor_copy(pt[:, :], T[:, c::4])
                        pls.append(pt)
                    w, x, y, z = pls

                    s1 = tp.tile([P, M], BF16, tag="tmp", name=f"s1{side}{it}")
                    s2 = tp.tile([P, M], BF16, tag="tmp", name=f"s2{side}{it}")
                    nc.vector._custom_dve(SQSUM, out=s1[:, :], in0=w[:, :], in1=z[:, :], s0=0.5)
                    nc.vector._custom_dve(SQSUM, out=s2[:, :], in0=x[:, :], in1=y[:, :], s0=0.5)
                    u3 = up.tile([P, M], BF16, tag="u", name=f"u3{side}{it}")
                    n2 = up.tile([P, M], BF16, tag="u", name=f"n2{side}{it}")
                    nc.vector.tensor_sub(u3[:, :], s1[:, :], s2[:, :])
                    nc.vector.tensor_add(n2[:, :], s1[:, :], s2[:, :])

                    wy = tp.tile([P, M], BF16, tag="tmp", name=f"wy{side}{it}")
                    xz = tp.tile([P, M], BF16, tag="tmp", name=f"xz{side}{it}")
                    yz = tp.tile([P, M], BF16, tag="tmp", name=f"yz{side}{it}")
                    wx = tp.tile([P, M], BF16, tag="tmp", name=f"wx{side}{it}")
                    nc.vector.tensor_mul(wy[:, :], w[:, :], y[:, :])
                    nc.vector.tensor_mul(xz[:, :], x[:, :], z[:, :])
                    nc.vector.tensor_mul(yz[:, :], y[:, :], z[:, :])
                    nc.vector.tensor_mul(wx[:, :], w[:, :], x[:, :])
                    u1 = up.tile([P, M], BF16, tag="u", name=f"u1{side}{it}")
                    u2 = up.tile([P, M], BF16, tag="u", name=f"u2{side}{it}")
                    nc.vector.tensor_add(u1[:, :], wy[:, :], xz[:, :])
                    nc.vector.tensor_sub(u2[:, :], yz[:, :], wx[:, :])
                    us[side] = (u1, u2, u3, n2)

                u1p, u2p, u3p, n2p = us["p"]
                u1t, u2t, u3t, n2t = us["t"]

                g = tp.tile([P, M], BF16, tag="tmp", name=f"g{it}")
                d1 = tp.tile([P, M], BF16, tag="tmp", name=f"d1{it}")
                d2 = tp.tile([P, M], BF16, tag="tmp", name=f"d2{it}")
                d3 = tp.tile([P, M], BF16, tag="tmp", name=f"d3{it}")
                nc.vector.tensor_mul(g[:, :], n2p[:, :], n2t[:, :])
                nc.vector.tensor_mul(d1[:, :], u1p[:, :], u1t[:, :])
                nc.vector.tensor_mul(d2[:, :], u2p[:, :], u2t[:, :])
                nc.vector.tensor_mul(d3[:, :], u3p[:, :], u3t[:, :])
                dd = tp.tile([P, M], BF16, tag="tmp", name=f"dd{it}")
                d = tp.tile([P, M], BF16, tag="tmp", name=f"d{it}")
                nc.vector.tensor_add(dd[:, :], d1[:, :], d2[:, :])
                nc.vector.tensor_add(d[:, :], dd[:, :], d3[:, :])

                m1 = tp.tile([P, M], BF16, tag="tmp", name=f"m1{it}")
                m2 = tp.tile([P, M], BF16, tag="tmp", name=f"m2{it}")
                nc.vector._custom_dve(M1C, out=m1[:, :], in0=g[:, :], in1=d[:, :], s0=TINY)
                nc.vector._custom_dve(M2C, out=m2[:, :], in0=g[:, :], in1=d[:, :])
                rho = tp.tile([P, M], BF16, tag="tmp", name=f"rho{it}")
                nc.vector._custom_dve(
                    RHORECIP, out=rho[:, :], in0=m2[:, :], in1=m1[:, :],
                    s0=rc["s0"], s1=rc["s1"],
                )
                t = tp.tile([P, M], BF16, tag="tmp", name=f"t{it}")
                nc.scalar.activation(t[:, :], rho[:, :], AF.Sqrt)

                phi = tp.tile([P, M], BF16, tag="tmp", name=f"phi{it}")
                nc.vector._custom_dve(
                    ATAN7, out=phi[:, :], in0=t[:, :], s0=A1, s1=A2, imm2=A3
                )

                # stats1[:, it] = sum(phi^2)
                ph2 = jkp.tile([P, M], BF16, tag="jk", name=f"ph2{it}")
                nc.scalar.activation(
                    ph2[:, :], phi[:, :], AF.Square,
                    accum_out=stats1[:, it : it + 1],
                )
                # stats2[:, it] = sum([d<0] * (pi^2 - 4*pi*phi))
                cjk = jkp.tile([P, M], BF16, tag="jk", name=f"cjk{it}")
                nc.vector._custom_dve(
                    CONDACC,
                    out=cjk[:, :],
                    accum_out=stats2[:, it : it + 1],
                    in0=d[:, :],
                    in1=phi[:, :],
                    s0=PI * PI,
                    s1=4.0 * PI,
                )

            nc.sync.dma_start(out=out[:, 0:NIT], in_=stats1[:, :])
            nc.sync.dma_start(out=out[:, NIT : 2 * NIT], in_=stats2[:, :])
    return nc


_CACHE = {}


def _get_nc(reps=1):
    key = ("nc", reps)
    if key not in _CACHE:
        nc = _emit(bacc.Bacc(), reps=reps)
        nc.compile()
        _CACHE[key] = nc
    return _CACHE[key]


def kernel(pred: np.ndarray, target: np.ndarray) -> np.ndarray:
    pred = np.ascontiguousarray(pred, dtype=np.float32)
    target = np.ascontiguousarray(target, dtype=np.float32)
    assert pred.shape == (B, 4) and target.shape == (B, 4)

    nc = _get_nc()
    in_maps = []
    for c in range(NCORES):
        sl = slice(c * QPC, (c + 1) * QPC)
        in_maps.append(
            {
                "pred": pred[sl].reshape(P, QPP * 4),
                "target": target[sl].reshape(P, QPP * 4),
            }
        )
    res = run_bass_kernel_spmd(nc, in_maps, list(range(NCORES)))
    total = 0.0
    for r in res.results:
        o = np.asarray(r["out"], np.float64)
        total += 4.0 * o[:, 0:NIT].sum() + o[:, NIT:].sum()
    return np.float32(total / B)
